# revision 62
# baseline (speedup 1.0000x reference)
"""LocalityEnhancedAttention Trainium2 kernel (8 NeuronCores, SPMD).

Sharding: core c handles batch b = c//2 and head-group g = c%2
(8 of 16 heads). Each core computes its partial output projection
(attn_heads @ wo_shard.T); host sums the two head-group partials per
batch and adds bo + wo@bv (the V-bias folds out of the device entirely
because softmax rows sum to one; the K-bias shifts every logit in a row
equally so it cancels in softmax; the Q-bias is added during the Q
projection's PSUM->SBUF copy as a per-partition DVE scalar-add).

The kernel is a single software pipeline paced by the scalar engine's
exp stream (the roofline engine: 256 x [128,1024] Exp ~ 285us):
  - lead-in: only K-proj chunk 0 + Q-proj(ch0, pt0), so the first
    scores/exp fire ~15us in;
  - the remaining K chunks and all V-proj groups are emitted lock-step
    inside (ch0, pt0)'s kj loop right behind each scores matmul;
  - one Q-proj job is popped at every pair boundary to give the tensor
    queue independent work while the softmax-normalize chain drains;
  - PV lags scores by one kj tile; head pairs are packed into PE
    row-halves (tile_position), which the PE executes concurrently.

Device dataflow per core (S=2048, M=1024, local head-dims DH=512):
  scores^T[kj, qi] = K^T.T @ Q^T per head -> exp via ACT (scale=1/8) ->
  banded local bias applied multiplicatively on bf16 probs (DVE) ->
  PV: A^T_aug[65, qi] += V_aug[kj].T @ P^T[kj] in PSUM (row 64 =
  softmax denominators) -> normalize (batched column-bound reciprocal +
  gpsimd partition_broadcast + DVE mul) -> WO -> bf16 out.
"""

import os
import sys
from contextlib import ExitStack

import numpy as np

sys.path.insert(0, "/opt/trn_rl_repo")

import ml_dtypes

BF = ml_dtypes.bfloat16

import concourse.bass as bass
import concourse.mybir as mybir
import concourse.tile as tile
from concourse import bacc
from concourse.bass_utils import run_bass_kernel_spmd

F32 = mybir.dt.float32
BF16 = mybir.dt.bfloat16
EXP = mybir.ActivationFunctionType.Exp
TS, DS = bass.ts, bass.ds

S = 2048
M = 1024
DH = 512        # head dims per core (8 heads x 64)
DK = 64
W = 16
NPT = 4         # head pairs per core
NCH = 4         # qi chunks of 512
NKJ = 16        # kj tiles of 128


def _emit(ctx, tc, io):
    nc = tc.nc

    const = ctx.enter_context(tc.tile_pool(name="const", bufs=1))
    qkvp = ctx.enter_context(tc.tile_pool(name="qkv", bufs=1))
    ap_ = ctx.enter_context(tc.tile_pool(name="anorm", bufs=1))
    wop = ctx.enter_context(tc.tile_pool(name="wop", bufs=1))

    qT_sb = [qkvp.tile([128, S], BF16, tag=f"q{i}", name=f"q{i}") for i in range(NPT)]
    kT_sb = [qkvp.tile([128, S], BF16, tag=f"k{i}", name=f"k{i}") for i in range(NPT)]
    v_sb = [qkvp.tile([128, 8 * 65], BF16, tag=f"v{i}", name=f"v{i}") for i in range(16)]
    a_sb = [ap_.tile([128, S], BF16, tag=f"a{i}", name=f"a{i}") for i in range(NPT)]

    # shared PSUM pool: projection accumulators AND the WO accumulators
    pp = ctx.enter_context(tc.tile_pool(name="pp", bufs=2, space="PSUM"))
    sps = ctx.enter_context(tc.tile_pool(name="sps", bufs=2, space="PSUM"))
    pvs = ctx.enter_context(tc.tile_pool(name="pvs", bufs=2, space="PSUM"))

    wpool = ctx.enter_context(tc.tile_pool(name="wpool", bufs=3))
    stream = ctx.enter_context(tc.tile_pool(name="stream", bufs=2))
    ptp = ctx.enter_context(tc.tile_pool(name="ptp", bufs=6))
    rp = ctx.enter_context(tc.tile_pool(name="rp", bufs=1))
    op = ctx.enter_context(tc.tile_pool(name="op", bufs=2))

    def proj_in(name, pts=None):
        # [1024, 512] HBM slab -> [128, 8, 512] SBUF tile; when pts is
        # given, only those head-pair column groups are fetched (the
        # rest arrives via proj_in_rest) so the lead-in DMA is slim
        t = wpool.tile([128, 8, DH], BF16, tag="w", name="w")
        src = io[name].rearrange("(k p) d -> p k d", k=8)
        if pts is None:
            nc.sync.dma_start(t[:], src)
        else:
            for pt in pts:
                nc.sync.dma_start(t[:, :, TS(pt, 128)], src[:, :, TS(pt, 128)])
        return t

    def proj_in_rest(t, name, pts):
        src = io[name].rearrange("(k p) d -> p k d", k=8)
        for pt in pts:
            nc.sync.dma_start(t[:, :, TS(pt, 128)], src[:, :, TS(pt, 128)])

    def x_chunk(name, sc, bufs):
        t = stream.tile([128, 8, 512], BF16, tag=f"s_{name}", name="s",
                        bufs=bufs)
        nc.sync.dma_start(
            t[:], io[name].rearrange("(k p) s -> p k s", k=8)[:, :, TS(sc, 512)])
        return t

    # ---- DMA schedule: ordered by first-use deadline ----
    wk = proj_in("wkT", pts=[0])
    xs_k = [None] * 4
    xs_k[0] = x_chunk("kT", 0, 4)
    wq = proj_in("wqT", pts=[0])
    xs_q = [None] * NCH
    xs_q[0] = x_chunk("qT", 0, 2)
    bqT = const.tile([128, NPT], F32, tag="bqT", name="bqT")
    nc.sync.dma_start(bqT[:], io["bqT"])
    pat = const.tile([128, 320], BF16, tag="pat", name="pat")
    nc.sync.dma_start(pat[:], io["pat"])
    xs_k[1] = x_chunk("kT", 1, 4)
    wv = proj_in("wvT")
    xs_v = [None] * 4
    xs_v[0] = x_chunk("vT", 0, 3)
    xs_k[2] = x_chunk("kT", 2, 4)
    xs_v[1] = x_chunk("vT", 1, 3)
    xs_k[3] = x_chunk("kT", 3, 4)
    xs_v[2] = x_chunk("vT", 2, 3)
    proj_in_rest(wk, "wkT", [1, 2, 3])
    proj_in_rest(wq, "wqT", [1, 2, 3])
    woT_sb = wop.tile([128, NPT, M], BF16, tag="wo", name="wo")
    nc.sync.dma_start(
        woT_sb[:], io["woT"].rearrange("(t p) m -> p t m", t=NPT))

    # PE p-state warm-up: ~4us of dependency-free matmuls on a zeroed
    # scratch tile so the systolic array reaches full clock while the
    # first input DMAs are still in flight.  `dummy()` re-uses the same
    # trick inside the steady loops: the PE only reaches its full 2.4GHz
    # clock after ~3us of CONTINUOUS busy, so dependency-free filler
    # matmuls around the known sub-us stall points keep the clock up
    # (the exp stream pace more than absorbs their cost).
    scr = const.tile([128, 512], BF16, tag="scr", name="scr")
    nc.vector.memset(scr[:], 0.0)
    def dummy(n=1):
        for _ in range(n):
            t = pp.tile([128, 512], F32, tag="pp", name="pp")
            nc.tensor.matmul(
                t[:], lhsT=scr[:, 0:128], rhs=scr[:],
                start=True, stop=True, skip_group_check=True)

    for _ in range(18):
        pswu = pp.tile([128, 512], F32, tag="pp", name="pp")
        nc.tensor.matmul(
            pswu[:], lhsT=scr[:, 0:128], rhs=scr[:],
            start=True, stop=True, skip_group_check=True)

    for st in range(16):
        vv = v_sb[st].rearrange("p (h e) -> p h e", e=65)
        nc.vector.memset(vv[:, :, 64:65], 1.0)

    # ---- projection group emitters (called from inside the pipeline) ----
    def k_sub(sc, pt, half=None, box=None):
        if half in (None, 0):
            psq = pp.tile([128, 512], F32, tag="pp", name="pp")
            if box is not None:
                box["psq"] = psq
        else:
            psq = box["psq"]
        ks = range(8) if half is None else range(half * 4, half * 4 + 4)
        for k in ks:
            nc.tensor.matmul(
                psq[:],
                lhsT=wk[:, k, TS(pt, 128)],
                rhs=xs_k[sc][:, k, :],
                start=(k == 0), stop=(k == 7), skip_group_check=True)
        if half in (None, 1):
            nc.vector.tensor_copy(kT_sb[pt][:, TS(sc, 512)], psq[:])

    def v_st(st):
        sc, j = st // 4, st % 4
        if j == 0 and sc + 1 < 4 and xs_v[sc + 1] is None:
            xs_v[sc + 1] = x_chunk("vT", sc + 1, 3)
        psv = pp.tile([128, DH], F32, tag="pp", name="pp")
        for k in range(8):
            nc.tensor.matmul(
                psv[:],
                lhsT=xs_v[sc][:, k, TS(j, 128)],
                rhs=wv[:, k, :],
                start=(k == 0), stop=(k == 7), skip_group_check=True)
        vv = v_sb[st].rearrange("p (h e) -> p h e", e=65)
        pv_view = psv.rearrange("p (h e) -> p h e", e=64)
        nc.vector.tensor_copy(vv[:, :, 0:64], pv_view[:, :, :])

    def q_proj_chunk(ch, pt, half=None, box=None):
        if half in (None, 0):
            psq = pp.tile([128, 512], F32, tag="pp", name="pp")
            if box is not None:
                box["psq"] = psq
        else:
            psq = box["psq"]
        ks = range(8) if half is None else range(half * 4, half * 4 + 4)
        for k in ks:
            nc.tensor.matmul(
                psq[:],
                lhsT=wq[:, k, TS(pt, 128)],
                rhs=xs_q[ch][:, k, :],
                start=(k == 0), stop=(k == 7), skip_group_check=True)
        if half in (None, 1):
            # copy + per-partition Q bias on DVE
            nc.vector.tensor_scalar_add(
                qT_sb[pt][:, TS(ch, 512)], psq[:], bqT[:, DS(pt, 1)])

    # ---- lead-in: just enough for the first scores/exp ----
    k_sub(0, 0)
    q_proj_chunk(0, 0)

    # one q-proj job per pair boundary, emitted ~2 boundaries before use
    qjobs = [(0, 2), (0, 3)] + [
        (c, p) for c in range(1, NCH) for p in range(NPT)]

    patv = pat.rearrange("p (h w) -> p h w", h=2)
    wo_pending = []    # deferred WO st-groups of the previous chunk
    kbox = {}          # open K-projection PSUM group (split k_sub)
    pv_pending = []    # previous pair's trailing PV steps
    post_pending = []  # previous pair's normalize emission
    norm_pending = []  # deferred normalize tail of the previous pair
    tensor_jobs = []   # deferred q-projections

    def emit_wo(ch, j, mt, box):
        st = ch * 4 + j
        if mt == 0:
            box["ot"] = op.tile([128, M], BF16, tag="ot", name="ot")
        ot = box["ot"]
        pso = pp.tile([128, 512], F32, tag="pp", name="pp")
        for pt in range(NPT):
            nc.tensor.matmul(
                pso[:],
                lhsT=a_sb[pt][:, TS(st, 128)],
                rhs=woT_sb[:, pt, TS(mt, 512)],
                start=(pt == 0), stop=(pt == 3),
                skip_group_check=True)
        if ch == NCH - 1 and mt == 0:
            # tail: ACT is idle after the last exp; use it for half the
            # casts so the final WO flush isn't paced by the DVE queue
            nc.scalar.copy(ot[:, TS(mt, 512)], pso[:])
        else:
            nc.vector.tensor_copy(ot[:, TS(mt, 512)], pso[:])
        if ch == NCH - 1:
            # last chunk: ship each half as soon as its cast lands
            nc.sync.dma_start(
                io["out"][TS(st, 128), TS(mt, 512)], ot[:, TS(mt, 512)])
        elif mt == 1:
            nc.sync.dma_start(io["out"][TS(st, 128), :], ot[:])

    for ch in range(NCH):
        if ch + 1 < NCH:
            xs_q[ch + 1] = x_chunk("qT", ch + 1, 2)
        c0 = ch * 512
        for pt in range(NPT):
            pvt = [pvs.tile([65, 512], F32, tag="pv", name="pv") for _ in range(2)]
            ptts = [None] * NKJ

            def pv_step(kj):
                for h in (0, 1):
                    lh = pt * 2 + h
                    nc.tensor.matmul(
                        pvt[h][:],
                        lhsT=v_sb[kj][:, DS(lh * 65, 65)],
                        rhs=ptts[kj][:, DS(h * 512, 512)],
                        start=(kj == 0), stop=(kj == 15),
                        skip_group_check=True)

            for kj in range(16):
                kj0 = kj * 128
                sp = sps.tile([128, 1024], F32, tag="sp", name="sp")
                for h in (0, 1):
                    nc.tensor.matmul(
                        sp[:, DS(h * 512, 512)],
                        lhsT=kT_sb[pt][DS(h * 64, 64), TS(kj, 128)],
                        rhs=qT_sb[pt][DS(h * 64, 64), TS(ch, 512)],
                        start=True, stop=True,
                        tile_position=(h * 64, 0),
                        skip_group_check=True)
                ptt = ptp.tile([128, 1024], BF16, tag="ptt", name="ptt")
                nc.scalar.activation(ptt[:], sp[:], EXP, scale=0.125)
                ptts[kj] = ptt
                lo = max(kj0 - W, c0)
                hi = min(kj0 + 128 + W, c0 + 512)
                if lo < hi:
                    pa = lo - (kj0 - W)
                    pv3 = ptt.rearrange("p (h w) -> p h w", h=2)
                    nc.vector.tensor_mul(
                        pv3[:, :, DS(lo - c0, hi - lo)],
                        pv3[:, :, DS(lo - c0, hi - lo)],
                        patv[:, :, DS(pa, hi - lo)])
                # deferred work rides inside the kj loops, right behind
                # the scores that feed the exp stream: the previous
                # pair's normalize tail (one DVE step per kj so the band
                # muls aren't starved), projections, and the previous
                # chunk's WO groups.
                if pv_pending:
                    # previous pair's trailing PV steps ride behind our
                    # first scores so its exps aren't delayed at the
                    # boundary
                    pv_pending.pop(0)()
                    if kj == 0 and pv_pending:
                        pv_pending.pop(0)()
                if post_pending and kj == 2:
                    # previous pair's normalize emission: must follow its
                    # deferred PV tail (readers are ordered by emission)
                    post_pending.pop(0)()
                if norm_pending and kj >= 3:
                    norm_pending.pop(0)()
                    if kj == 3 and norm_pending:
                        norm_pending.pop(0)()
                if tensor_jobs and kj in (5, 7):
                    tensor_jobs.pop(0)()
                if wo_pending and kj in (9, 11, 13, 15):
                    wo_pending.pop(0)()
                if ch == 0:
                    if pt == 0 and kj >= 1:
                        v_st(kj - 1)
                    if kj % 4 == 0 and kj // 4 + 1 < 4:
                        if pt == 0:
                            k_sub(kj // 4 + 1, pt)
                        else:
                            # halved so the insertion fits the 2-exp
                            # pipeline buffer without stalling the stream
                            kbox.clear()
                            k_sub(kj // 4 + 1, pt, 0, kbox)
                    if kj % 4 == 1 and kj // 4 + 1 < 4 and pt > 0:
                        k_sub(kj // 4 + 1, pt, 1, kbox)
                    if kj == 14 and pt < 3:
                        k_sub(0, pt + 1)
                    if kj == 15 and pt == 0:
                        q_proj_chunk(0, 1)
                if kj > 2:
                    pv_step(kj - 3)
            if ch == 0 and pt == 0:
                v_st(15)
            if ch == NCH - 1 and pt == NPT - 1:
                pv_step(13)
                pv_step(14)
                pv_step(15)
            else:
                def _pv_tail(k, _pvt=pvt, _ptts=ptts, _pt=pt):
                    for h in (0, 1):
                        nc.tensor.matmul(
                            _pvt[h][:],
                            lhsT=v_sb[k][:, DS((_pt * 2 + h) * 65, 65)],
                            rhs=_ptts[k][:, DS(h * 512, 512)],
                            start=False, stop=(k == 15),
                            skip_group_check=True)
                pv_pending.extend(
                    (lambda k=k, f=_pv_tail: f(k)) for k in (13, 14, 15))

            last = ch == NCH - 1 and pt == NPT - 1

            def boundary_tail(pvt=pvt, pt=pt, ch=ch, last=last):
                # queue the next q-projection; it drains mid-way through
                # the next kj loop so it doesn't delay the next scores
                if qjobs:
                    cpt = qjobs.pop(0)
                    qbox = {}
                    tensor_jobs.extend(
                        (lambda cpt=cpt, hh=h2, bb=qbox:
                         q_proj_chunk(*cpt, half=hh, box=bb))
                        for h2 in range(2))

                # normalize. All PSUM reads happen in the copies up front
                # so the pvt banks free early; one column-bound recip
                # covers both heads (rows 0 / 32; offsets must be x32).
                # The recip/broadcast/mul tail drains one step per kj so
                # it doesn't block the band muls queued behind it on DVE.
                den2 = rp.tile([33, 512], F32, tag="den2", name="den2")
                if last:
                    # no next pair: normalize straight out of PSUM; keep
                    # the PE clock up so the WO flush runs fast
                    araw = [pvt[h][DS(0, 64), :] for h in (0, 1)]
                    dummy(50)
                else:
                    araw = [rp.tile([64, 512], F32, tag=f"ar{h}",
                                    name=f"ar{h}") for h in (0, 1)]
                    for h in (0, 1):
                        nc.vector.tensor_copy(
                            araw[h][:], pvt[h][DS(0, 64), :])
                for h in (0, 1):
                    nc.vector.tensor_copy(
                        den2[DS(h * 32, 1), :], pvt[h][DS(64, 1), :])
                rec2 = rp.tile([33, 512], F32, tag="rec2", name="rec2")
                r1b = rp.tile([1, 512], F32, tag="r1b", name="r1b")

                def _n_recip(den2=den2, rec2=rec2):
                    nc.vector.reciprocal(rec2[:], den2[:])

                def _n_rehome(rec2=rec2, r1b=r1b):
                    # partition_broadcast only reads partition 0, so
                    # re-home head 1's recip row to a partition-0 tile
                    nc.vector.tensor_copy(r1b[:], rec2[DS(32, 1), :])

                def _n_mul(h, rsrc, araw=araw):
                    # gpsimd runs ONLY partition_broadcast (mixing op
                    # types thrashes its library, ~6us load/unloads)
                    rb = rp.tile([64, 512], F32, tag="rb", name="rb",
                                 bufs=2)
                    nc.gpsimd.partition_broadcast(rb[:], rsrc)
                    nc.vector.tensor_mul(
                        a_sb[pt][DS(h * 64, 64), TS(ch, 512)],
                        araw[h][:, :], rb[:])

                steps = [
                    _n_recip,
                    _n_rehome,
                    lambda rec2=rec2: _n_mul(0, rec2[DS(0, 1), :]),
                    lambda r1b=r1b: _n_mul(1, r1b[:]),
                ]
                if last:
                    for fn in steps:
                        fn()
                else:
                    norm_pending.extend(steps)

            if last:
                boundary_tail()
            else:
                post_pending.append(boundary_tail)

        # WO for this chunk is deferred into the next chunk's pt0 loop
        # (emitted behind the scores there) so the next chunk's exp
        # stream starts immediately; the last chunk flushes here.
        assert not wo_pending
        for j in range(4):
            box = {}
            wo_pending.extend(
                (lambda cc, jj, mm, bb: lambda: emit_wo(cc, jj, mm, bb))(
                    ch, j, mt, box) for mt in range(2))
        if ch == NCH - 1:
            for fn in norm_pending:
                fn()
            norm_pending.clear()
            for fn in wo_pending:
                fn()
            wo_pending.clear()


_CACHE = {}


def _build():
    if "nc" in _CACHE:
        return _CACHE["nc"]
    nc = bacc.Bacc("TRN2", target_bir_lowering=False, debug=False)
    io = {}
    for name, shape in (
        ("qT", [M, S]), ("kT", [M, S]), ("vT", [M, S]),
        ("wqT", [M, DH]), ("wkT", [M, DH]), ("wvT", [M, DH]),
        ("woT", [DH, M]),
    ):
        io[name] = nc.dram_tensor(name, shape, BF16, kind="ExternalInput").ap()
    io["pat"] = nc.dram_tensor("pat", [128, 320], BF16, kind="ExternalInput").ap()
    io["bqT"] = nc.dram_tensor("bqT", [128, NPT], F32, kind="ExternalInput").ap()
    io["out"] = nc.dram_tensor("out", [S, M], BF16, kind="ExternalOutput").ap()
    with tile.TileContext(nc) as tc:
        with ExitStack() as ctx:
            _emit(ctx, tc, io)
    nc.compile()
    _CACHE["nc"] = nc
    return nc


def _bias_pattern(local_bias):
    # multiplicative band pattern: exp(2*b[qi-kj+W]) inside the band, 1.0
    # outside; duplicated side by side for the two heads of a pair tile.
    p = np.arange(128)[:, None]
    f = np.arange(160)[None, :]
    idx = f - p  # rel + W
    valid = (idx >= 0) & (idx <= 2 * W)
    b = np.asarray(local_bias, np.float64)
    pat = np.where(valid, np.exp(2.0 * b[np.clip(idx, 0, 2 * W)]), 1.0)
    pat2 = np.concatenate([pat, pat], axis=1)
    return np.ascontiguousarray(pat2).astype(BF)


def kernel(query, key, value, wq, bq, wk, bk, wv, bv, wo, bo, local_bias):
    query = np.asarray(query, np.float32)
    key = np.asarray(key, np.float32)
    value = np.asarray(value, np.float32)
    wq, wk, wv, wo = (np.asarray(x, np.float32) for x in (wq, wk, wv, wo))
    bq, bk, bv, bo = (np.asarray(x, np.float32) for x in (bq, bk, bv, bo))
    pat = _bias_pattern(local_bias)

    nc = _build()
    in_maps = []
    for c in range(8):
        b, g = c // 2, c % 2
        sl = slice(g * DH, (g + 1) * DH)
        # bq shard laid out [128, NPT]: column pt = bias dims pt*128..+128
        bq_t = np.ascontiguousarray(
            bq[sl].reshape(NPT, 128).T).astype(np.float32)
        in_maps.append({
            "qT": np.ascontiguousarray(query[b].T).astype(BF),
            "kT": np.ascontiguousarray(key[b].T).astype(BF),
            "vT": np.ascontiguousarray(value[b].T).astype(BF),
            "wqT": np.ascontiguousarray(wq[sl, :].T).astype(BF),
            "wkT": np.ascontiguousarray(wk[sl, :].T).astype(BF),
            "wvT": np.ascontiguousarray(wv[sl, :].T).astype(BF),
            "woT": np.ascontiguousarray(wo[:, sl].T).astype(BF),
            "bqT": bq_t,
            "pat": pat,
        })
    res = run_bass_kernel_spmd(
        nc, in_maps, core_ids=list(range(8)),
        trace=bool(int(os.environ.get("KERNEL_TRACE", "0"))),
    )
    _CACHE["last_result"] = res
    outs = [np.asarray(r["out"], np.float32) for r in res.results]
    # host folds: V-bias (softmax rows sum to 1) and the output bias
    cvec = (wo @ bv + bo).astype(np.float32)
    out = np.stack([outs[2 * b] + outs[2 * b + 1] + cvec for b in range(4)])
    return out.astype(np.float32)
